# revision 44
# baseline (speedup 1.0000x reference)
"""LSTM warmup + autoregressive decode kernel for Trainium2 (Bass/Tile).

Reference computation (per batch row):
  h,c = 0
  for t in range(T):  h,c = LSTMstep(x_t)        # warmup over input seq
  pred0 = h @ Wd + bd
  for d in range(out_steps-1): h,c = LSTMstep(pred_d); pred_{d+1} = h@Wd+bd
  out[b, s, f] = pred_s

Strategy: data-parallel over 8 NeuronCores (B=4096 -> 512/core); the
sequential time loop stays local per shard.  The 512-row shard is further
split into TWO independent 256-row streams whose LSTM recurrences interleave:
while stream A sits in its serial step boundary (h-mul -> recurrence matmuls
-> first gate activation), the Activation engine processes stream B's gates,
keeping the bottleneck engine (Act) near-saturated.

On-chip layout is transposed (partitions = unit index within a 128-chunk,
free dim = (unit-chunk, batch)): z per (stream, gate) is a [128, 512] PSUM
tile (= one bank; cols = 2 unit-chunks x 256 batch), activated in one wide
Act op; gate/h tiles are bf16 so DVE elementwise runs in 2x perf mode and h
feeds the next step's matmuls directly with no transposes.

Bias handling: warmup x tiles are [65, 512] with a constant ones row and the
stationary W carries a bias row, so z picks up +b inside the x-matmul and the
wide activations need no per-partition bias operand (which could not express
a bias differing between the two unit-chunk column halves).  Decode (no
x-pass) initializes each accumulation with a K=1 ones-row matmul carrying the
fused decode bias.

The autoregressive decode is algebraically fused: since
  z_{t+1} = pred_t @ W + h_t @ U + b   and   pred_t = h_t @ Wd + bd,
we precompute Ud = U + Wd@W and bdec = b + bd@W on the host, so each decode
step is a single K=256 recurrence; pred is computed off the critical path
directly in [batch, feature] layout (h as the stationary operand), so the
output path needs no transposes.
"""

import sys

for _p in ("/opt/trn_rl_repo", "/root/.axon_site/_ro/trn_rl_repo"):
    if _p not in sys.path:
        sys.path.insert(0, _p)

import numpy as np

import concourse.bacc as bacc
import concourse.mybir as mybir
import concourse.tile as tile
from concourse import bass_utils

F32 = mybir.dt.float32
BF16 = mybir.dt.bfloat16
AF = mybir.ActivationFunctionType

N_CORES = 8
F = 64          # input/output feature dim
U = 256         # lstm units
U4 = 4 * U      # gate rows
XLOOK = 3       # steps of x-staging lookahead
NXS = 6         # static x tiles
NS = 2          # batch streams per core
SB = 256        # stream batch

G_F, G_I, G_G, G_O = 0, 1, 2, 3
# column base in the 1024-wide gate row space, keras order (i, f, g, o)
GCOL = {G_I: 0, G_F: 256, G_G: 512, G_O: 768}
CDT = BF16  # cell-state dtype (F32 for extra precision margin)


def build_program(B, T, out_steps):
    """Single-core SPMD program for a batch shard of size B (=512)."""
    assert B == 512, "tile geometry is hardcoded for a 512-row shard"
    NB = B // 128

    nc = bacc.Bacc("TRN2", target_bir_lowering=False, debug=False, num_devices=1)

    xin = nc.dram_tensor("xin", [B, T, F], F32, kind="ExternalInput").ap()
    wb_d = nc.dram_tensor("wb", [F + 1, U4], BF16, kind="ExternalInput").ap()
    u2_d = nc.dram_tensor("u2", [128, 2 * U4], BF16, kind="ExternalInput").ap()
    ud2_d = nc.dram_tensor("ud2", [128, 2 * U4], BF16, kind="ExternalInput").ap()
    wdd2_d = nc.dram_tensor("wdd2", [128, 2 * F], BF16, kind="ExternalInput").ap()
    bdec_d = nc.dram_tensor("bdec", [1, U4], BF16, kind="ExternalInput").ap()
    bdrow_d = nc.dram_tensor("bdrow", [1, F], BF16, kind="ExternalInput").ap()
    ones_d = nc.dram_tensor("ones", [1, SB], BF16, kind="ExternalInput").ap()
    ident_d = nc.dram_tensor("ident", [128, 128], F32, kind="ExternalInput").ap()
    yout = nc.dram_tensor("yout", [B, out_steps, F], F32, kind="ExternalOutput").ap()

    xin_f = xin.rearrange("b t f -> b (t f)")
    xin_c = xin.rearrange("(c p) t f -> p c (t f)", c=4)   # [128, 4, T*F]
    yout_f = yout.rearrange("b s f -> b (s f)")
    yout_c = yout.rearrange("(c p) s f -> p c (s f)", c=4)  # [128, 4, S*F]

    with tile.TileContext(nc) as tc:
        import contextlib

        with contextlib.ExitStack() as ctx:
            wpool = ctx.enter_context(tc.tile_pool(name="wpool", bufs=1))
            xspool = ctx.enter_context(tc.tile_pool(name="xspool", bufs=1))
            dpool = ctx.enter_context(tc.tile_pool(name="dpool", bufs=16))
            gpool = ctx.enter_context(tc.tile_pool(name="gpool", bufs=2))
            tpool = ctx.enter_context(tc.tile_pool(name="tpool", bufs=2))
            cpool = ctx.enter_context(tc.tile_pool(name="cpool", bufs=2))
            hpool = ctx.enter_context(tc.tile_pool(name="hpool", bufs=3))
            opool = ctx.enter_context(tc.tile_pool(name="opool", bufs=4))
            zpool = ctx.enter_context(tc.tile_pool(name="zpool", bufs=6, space="PSUM"))
            aux = ctx.enter_context(tc.tile_pool(name="aux", bufs=2, space="PSUM"))

            # ---- weights / constants ----
            # (ident + wb first: step 0 needs only those; the big u2/ud2
            # transfers go after the prologue x staging, see below)
            ident = wpool.tile([128, 128], F32)
            wb = wpool.tile([F + 1, U4], BF16)
            ones = wpool.tile([1, SB], BF16)
            u2 = wpool.tile([128, 2 * U4], BF16)
            ud2 = wpool.tile([128, 2 * U4], BF16)
            wdd2 = wpool.tile([128, 2 * F], BF16)
            bdec = wpool.tile([1, U4], BF16)
            bdrow = wpool.tile([1, F], BF16)

            # static x tiles: rows 0:64 = x_t^T (bf16), row 64 = ones
            # (cols st*SB:(st+1)*SB belong to stream st)
            xs = [xspool.tile([F + 1, B], BF16, name=f"xs{j}") for j in range(NXS)]
            for j in range(NXS):
                nc.gpsimd.memset(xs[j][F : F + 1, :], 1.0)

            # ---- x staging: DMA 4 batch-chunks, PE-transpose, Pool-copy ----
            def stage_x_dma(t):
                # all 4 batch-chunks of step t in one DMA: dt[p, bc*F+f]
                dt_in = dpool.tile([128, NB * F], F32, tag="din", name=f"din{t}")
                nc.sync.dma_start(
                    dt_in[:].rearrange("p (c f) -> p c f", c=NB),
                    xin_c[:, :, F * t : F * (t + 1)],
                )
                return dt_in

            def stage_x_transpose(t, dt_in):
                xp = aux.tile([128, B], F32, tag="aux", name=f"xp{t}")
                for bc in range(NB):
                    nc.tensor.transpose(
                        xp[0:F, 128 * bc : 128 * (bc + 1)],
                        dt_in[:, F * bc : F * (bc + 1)], ident[:],
                    )
                # Pool/GPSIMD cannot read PSUM on real HW; copy on DVE
                nc.vector.tensor_copy(xs[t % NXS][0:F, :], xp[0:F, :])

            # ---- per-(stream, step) PE pass emission ----
            # PSUM accumulation groups have 2KB-bank ("zero region")
            # granularity: the two column-half groups of a gate tile must run
            # SEQUENTIALLY (half 0's start..stop fully before half 1 starts).
            def emit_gate(t, st, zt, x_t, q, uw, h_prev, first):
                zq = zpool.tile([128, 2 * SB], F32, tag="z", name=f"z{t}_{st}_{q}")
                zt[q] = zq
                for hcol in (0, 1):
                    mcol = GCOL[q] + 128 * hcol
                    dst = zq[:, SB * hcol : SB * (hcol + 1)]
                    if x_t is not None:
                        nc.tensor.matmul(
                            dst, wb[:, mcol : mcol + 128],
                            x_t[:, SB * st : SB * (st + 1)],
                            start=True, stop=first,
                        )
                    else:
                        nc.tensor.matmul(
                            dst, bdec[:, mcol : mcol + 128], ones[:],
                            start=True, stop=first,
                        )
                    if not first:
                        nc.tensor.matmul(
                            dst, uw[:, mcol : mcol + 128], h_prev[:, 0:SB],
                            start=False, stop=False,
                        )
                        nc.tensor.matmul(
                            dst, uw[:, U4 + mcol : U4 + mcol + 128],
                            h_prev[:, SB : 2 * SB],
                            start=False, stop=True,
                        )

            def emit_acts(t, st, zt, c_prev, g_t):
                """Act ops f,i,g,o for one stream (tc emitted in emit_dve)."""
                nc.scalar.activation(g_t["f"][:], zt[G_F][:], AF.Sigmoid)
                nc.scalar.activation(g_t["i"][:], zt[G_I][:], AF.Sigmoid)
                nc.scalar.activation(g_t["g"][:], zt[G_G][:], AF.Tanh)
                nc.scalar.activation(g_t["o"][:], zt[G_O][:], AF.Sigmoid)

            def emit_dve(t, st, c_prev, g_t):
                """Elementwise chain + tanh(c) + h for one stream."""
                m = tpool.tile([128, 2 * SB], BF16, tag="m", name=f"m{t}_{st}")
                fc = tpool.tile([128, 2 * SB], CDT, tag="fc", name=f"fc{t}_{st}")
                c_t = cpool.tile([128, 2 * SB], CDT, tag="c", name=f"c{t}_{st}")
                tc_t = gpool.tile([128, 2 * SB], BF16, tag="tc", name=f"tc{t}_{st}")
                h_t = hpool.tile([128, 2 * SB], BF16, tag="h", name=f"h{t}_{st}")

                if c_prev is not None:
                    nc.vector.tensor_mul(fc[:], g_t["f"][:], c_prev[:])
                nc.vector.tensor_mul(m[:], g_t["i"][:], g_t["g"][:])
                if c_prev is not None:
                    nc.vector.tensor_add(c_t[:], fc[:], m[:])
                else:
                    nc.vector.tensor_copy(c_t[:], m[:])
                nc.scalar.activation(tc_t[:], c_t[:], AF.Tanh)
                # h in unit-chunk halves: the next step's first u-pass only
                # needs cols 0:SB, so it can start one sem-hop earlier
                nc.vector.tensor_mul(h_t[:, 0:SB], g_t["o"][:, 0:SB], tc_t[:, 0:SB])
                nc.vector.tensor_mul(
                    h_t[:, SB : 2 * SB], g_t["o"][:, SB : 2 * SB], tc_t[:, SB : 2 * SB]
                )
                return h_t, c_t

            # ---- pred + output (per stream: 2 batch chunks of 128) ----
            def emit_pred_mm(s, st, h_t):
                pp = aux.tile([128, B], F32, tag="aux", name=f"pp{s}_{st}")
                for j in range(2):
                    dst = pp[:, F * j : F * (j + 1)]
                    nc.tensor.matmul(
                        dst, ones[0:1, 0:128], bdrow[:], start=True, stop=False
                    )
                    nc.tensor.matmul(
                        dst, h_t[:, 128 * j : 128 * (j + 1)], wdd2[:, 0:F],
                        start=False, stop=False,
                    )
                    nc.tensor.matmul(
                        dst, h_t[:, SB + 128 * j : SB + 128 * (j + 1)],
                        wdd2[:, F : 2 * F],
                        start=False, stop=True,
                    )
                return pp

            def emit_pred_out(s, st, pp):
                osb = opool.tile([128, 2 * F], F32, tag="ot", name=f"osb{s}_{st}")
                nc.vector.tensor_copy(osb[:], pp[:, 0 : 2 * F])
                nc.sync.dma_start(
                    yout_c[:, 2 * st : 2 * st + 2, F * s : F * (s + 1)],
                    osb[:].rearrange("p (c f) -> p c f", c=2),
                )

            # ---- prologue: stage x for the first steps ----
            dma_q = {}
            dma_q[0] = stage_x_dma(0)
            nc.sync.dma_start(ident[:], ident_d[:])
            nc.sync.dma_start(wb[:], wb_d[:])
            for t in range(1, min(XLOOK, T)):
                dma_q[t] = stage_x_dma(t)
            # big weight transfers after the first x tiles are in flight
            nc.sync.dma_start(u2[:], u2_d[:])
            nc.sync.dma_start(ones[:], ones_d[:])
            nc.sync.dma_start(ud2[:], ud2_d[:])
            nc.sync.dma_start(wdd2[:], wdd2_d[:])
            nc.sync.dma_start(bdec[:], bdec_d[:])
            nc.sync.dma_start(bdrow[:], bdrow_d[:])
            for t in range(min(XLOOK, T)):
                stage_x_transpose(t, dma_q.pop(t))
            if XLOOK < T:
                dma_q[XLOOK] = stage_x_dma(XLOOK)

            n_steps = T + (out_steps - 1)
            h_prev = [None] * NS
            c_prev = [None] * NS

            hs = {}
            for t in range(n_steps):
                warm = t < T
                x_t = xs[t % NXS] if warm else None
                uw = u2 if warm else ud2
                first = h_prev[0] is None

                # --- PE: recurrence blocks per stream ---
                zt = [dict() for _ in range(NS)]
                for st in range(NS):
                    for q in (G_F, G_I, G_G, G_O):
                        emit_gate(t, st, zt[st], x_t, q, uw, h_prev[st], first)

                # pred matmuls for the previous step's h (decode lags 1 step)
                if t >= T and (t - 1) in hs:
                    pps = [emit_pred_mm(t - T, st, hs[t - 1][st]) for st in range(NS)]

                # --- x staging for upcoming steps ---
                if t + 1 < n_steps:
                    if t + XLOOK < T and t + XLOOK in dma_q:
                        stage_x_transpose(t + XLOOK, dma_q.pop(t + XLOOK))
                    if t + XLOOK + 1 < T:
                        dma_q[t + XLOOK + 1] = stage_x_dma(t + XLOOK + 1)

                # --- Act/DVE tails, stream-interleaved ---
                g_ts = []
                for st in range(NS):
                    g_t = {
                        k: gpool.tile(
                            [128, 2 * SB], BF16, tag=f"g{k}", name=f"g{k}{t}_{st}"
                        )
                        for k in ("f", "i", "g", "o")
                    }
                    g_ts.append(g_t)
                    emit_acts(t, st, zt[st], c_prev[st], g_t)
                    h_prev[st], c_prev[st] = emit_dve(t, st, c_prev[st], g_t)

                # output DMA for lagged preds
                if t >= T and (t - 1) in hs:
                    for st in range(NS):
                        emit_pred_out(t - T, st, pps[st])
                    del hs[t - 1]
                if t >= T - 1:
                    hs[t] = list(h_prev)

            # epilogue: last pred
            for st in range(NS):
                pp = emit_pred_mm(out_steps - 1, st, hs[n_steps - 1][st])
                emit_pred_out(out_steps - 1, st, pp)

    nc.compile()
    return nc


_CACHE = {}


def _get_program(key):
    if key not in _CACHE:
        _CACHE[key] = build_program(*key)
    return _CACHE[key]


def _host_prep(W, Uk, b, Wd, bd):
    bf16 = mybir.dt.np(BF16)
    W64 = W.astype(np.float64)
    Ud = (Uk.astype(np.float64) + Wd.astype(np.float64) @ W64).astype(np.float32)
    bdec = (b.astype(np.float64) + bd.astype(np.float64) @ W64).astype(np.float32)
    wb = np.concatenate([W, b.reshape(1, -1)], axis=0)          # [65, 1024]
    u2 = np.concatenate([Uk[0:128], Uk[128:256]], axis=1)       # [128, 2048]
    ud2 = np.concatenate([Ud[0:128], Ud[128:256]], axis=1)
    wdd2 = np.concatenate([Wd[0:128], Wd[128:256]], axis=1)     # [128, 128]
    return {
        "wb": wb.astype(bf16),
        "u2": u2.astype(bf16),
        "ud2": ud2.astype(bf16),
        "wdd2": wdd2.astype(bf16),
        "bdec": bdec.reshape(1, -1).astype(bf16),
        "bdrow": bd.reshape(1, -1).astype(bf16),
        "ones": np.ones((1, SB), dtype=bf16),
        "ident": np.eye(128, dtype=np.float32),
    }


def kernel(inputs, W, U, b, Wd, bd, out_steps):
    inputs = np.asarray(inputs, dtype=np.float32)
    W = np.asarray(W, dtype=np.float32)
    U_ = np.asarray(U, dtype=np.float32)
    b_ = np.asarray(b, dtype=np.float32)
    Wd = np.asarray(Wd, dtype=np.float32)
    bd = np.asarray(bd, dtype=np.float32)
    out_steps = int(out_steps)

    B_full, T, _ = inputs.shape
    assert B_full % N_CORES == 0
    Bc = B_full // N_CORES

    nc = _get_program((Bc, T, out_steps))
    shared = _host_prep(W, U_, b_, Wd, bd)
    in_maps = [
        {"xin": np.ascontiguousarray(inputs[i * Bc : (i + 1) * Bc]), **shared}
        for i in range(N_CORES)
    ]
    res = bass_utils.run_bass_kernel_spmd(nc, in_maps, core_ids=list(range(N_CORES)))
    out = np.concatenate([res.results[i]["yout"] for i in range(N_CORES)], axis=0)
    return out
